# revision 12
# baseline (speedup 1.0000x reference)
"""EvoBinarizedLayer as one fp8 matmul per population member.

Math: per population p, with xb = unpacked bits of x (LSB-first) and
w0/w1 the two unpacked weight bit-planes,

  count[p] = xb @ w0 + (1 - xb) @ w1
           = xb @ (w0 - w1) + colsum(w1)

so each core computes a single [512,2048] @ [2048,2048] matmul with
lhs entries in {0,1} and rhs entries in {-1,0,1} (both exact in fp8
e4m3, accumulated exactly in fp32 PSUM), plus a per-(p,o) bias added
on the host. Counts <= 2048 are exact in fp16, so the device emits
fp16 and the host upcasts to int32.

Sharding: population dim P=8, one member per NeuronCore (x replicated).

Device layout: contraction dim K=2048 split into 16 k-tiles of 128
(partition dim); DoubleRow fp8 matmuls consume k-tile pairs (K=256 per
instruction). Weights are streamed in 16 chunks of (k-quarter x
o-quarter), each chunk contiguous per partition so every DMA is 128
descriptors of 2 KiB, spread round-robin over 4 engine DMA queues.
"""

import numpy as np
import ml_dtypes

POP, BATCH, IN_INTS, OUT_F = 8, 512, 32, 2048
K = IN_INTS * 64          # 2048 contraction (bit) dim
KT = K // 128             # 16 k-tiles of 128
N_CORES = 8

_FP8 = ml_dtypes.float8_e4m3

_cached = {}

# matmul loop order: "A" = (ob, bt) outer, k inner (stationary reloaded
# every matmul); "B" = (bt, k) outer, ob inner (stationary repeated 4x
# consecutively, tests walrus LDWEIGHTS dedupe)
MM_ORDER = "A"


def _build_nc():
    import concourse.tile as tile
    from concourse import bacc, mybir

    dt = mybir.dt
    nc = bacc.Bacc(
        "TRN2", target_bir_lowering=False, debug=False, num_devices=N_CORES
    )
    xbt_d = nc.dram_tensor(
        "xbt", [4, 128, 4, BATCH], dt.float8e4, kind="ExternalInput"
    ).ap()
    wd_d = nc.dram_tensor(
        "wd", [4, 4, 128, 4, 512], dt.float8e4, kind="ExternalInput"
    ).ap()
    out_d = nc.dram_tensor(
        "out", [BATCH, OUT_F], dt.float16, kind="ExternalOutput"
    ).ap()

    with tile.TileContext(nc) as tc:
        with (
            tc.tile_pool(name="xbt", bufs=1) as xbt_pool,
            tc.tile_pool(name="wd", bufs=1) as wd_pool,
            tc.tile_pool(name="outp", bufs=4) as out_pool,
            tc.tile_pool(name="psum", bufs=8, space="PSUM") as psum_pool,
        ):
            engines = [nc.sync, nc.scalar, nc.gpsimd]
            rr = [0]

            def next_engine():
                e = engines[rr[0] % len(engines)]
                rr[0] += 1
                return e

            # xbt_sb[p, k, b]: bit row k*128+p, batch b
            xbt_sb = xbt_pool.tile([128, KT, BATCH], dt.float8e4)
            # wd_sb[p, ob, k, o']: bit row k*128+p, out feature ob*512+o'
            wd_sb = wd_pool.tile([128, 4, KT, 512], dt.float8e4)

            # PE warmup: dummy DoubleRow matmuls on a small zeroed tile so
            # the HAM clock-gate opens (K=8/8) before the real stream
            # starts. Small tile keeps the gating memset cheap (~0.3us).
            warm = xbt_pool.tile([128, 2, 128], dt.float8e4, tag="warm")
            nc.gpsimd.memset(warm[:], 0.0)
            wps = psum_pool.tile([128, 512], dt.float32, tag="ps")
            for _ in range(22):
                nc.tensor.matmul(
                    wps[:, :128],
                    warm[:],
                    warm[:],
                    start=True,
                    stop=True,
                    perf_mode=mybir.MatmulPerfMode.DoubleRow,
                )

            # input DMAs in need-order: the (ob=0) pass consumes chunk
            # pairs (xbt_kq, wd[kq,0]) in kq order; stream those first.
            issue = []
            for kq in range(4):
                issue.append(("x", kq))
                issue.append(("w", kq, 0))
            for ob in range(1, 4):
                for kq in range(4):
                    issue.append(("w", kq, ob))
            for item in issue:
                if item[0] == "x":
                    kq = item[1]
                    next_engine().dma_start(
                        xbt_sb[:, 4 * kq : 4 * (kq + 1), :], xbt_d[kq]
                    )
                else:
                    _, kq, ob = item
                    next_engine().dma_start(
                        wd_sb[:, ob, 4 * kq : 4 * (kq + 1), :], wd_d[kq, ob]
                    )

            # chunk-paced: within each o-quarter, sweep k-pairs in the
            # outer loop across 4 concurrent psum banks (one per batch
            # tile) so each arriving 256 KiB chunk feeds 8 matmuls before
            # the next chunk is needed (compute ramp matches DMA supply).
            # The final o-quarter instead runs tile-serial (k inner) so
            # its psum drains stagger and the last CAST+DMA tail is short.
            def drain(ps, ob, bt):
                ot = out_pool.tile([128, 512], dt.float16, tag="ot", name="ot")
                nc.scalar.copy(ot[:], ps[:])
                next_engine().dma_start(
                    out_d[128 * bt : 128 * (bt + 1), 512 * ob : 512 * (ob + 1)],
                    ot[:],
                )

            for ob in range(3):
                pss = [
                    psum_pool.tile(
                        [128, 512], dt.float32, tag="ps", name=f"ps_{ob}_{bt}"
                    )
                    for bt in range(4)
                ]
                for k in range(KT // 2):
                    for bt in range(4):
                        nc.tensor.matmul(
                            pss[bt][:],
                            xbt_sb[:, 2 * k : 2 * k + 2, 128 * bt : 128 * (bt + 1)],
                            wd_sb[:, ob, 2 * k : 2 * k + 2, :],
                            start=(k == 0),
                            stop=(k == KT // 2 - 1),
                            perf_mode=mybir.MatmulPerfMode.DoubleRow,
                        )
                for bt in range(4):
                    drain(pss[bt], ob, bt)
            for bt in range(4):
                ps = psum_pool.tile([128, 512], dt.float32, tag="ps", name="ps_l")
                for k in range(KT // 2):
                    nc.tensor.matmul(
                        ps[:],
                        xbt_sb[:, 2 * k : 2 * k + 2, 128 * bt : 128 * (bt + 1)],
                        wd_sb[:, 3, 2 * k : 2 * k + 2, :],
                        start=(k == 0),
                        stop=(k == KT // 2 - 1),
                        perf_mode=mybir.MatmulPerfMode.DoubleRow,
                    )
                drain(ps, 3, bt)
    nc.compile()
    return nc


def get_nc():
    if "nc" not in _cached:
        _cached["nc"] = _build_nc()
    return _cached["nc"]


def pack_inputs(x, w):
    """Host-side bit unpack + layout. Returns (xbt, wd_cores, bias).

    xbt: [4, 128, 4, BATCH] fp8; xbt[kq, p, k', b] = bit (4kq+k')*128+p of x[b]
    wd_cores[p]: [4, 4, 128, 4, 512] fp8; [kq, ob, p, k', o'] =
        (w0-w1) at bit row (4kq+k')*128+p, out feature ob*512+o'
    bias: [POP, OUT_F] int32 colsum of w1 bits
    """
    xb = np.unpackbits(
        x.view(np.uint8).reshape(BATCH, IN_INTS, 8), axis=-1, bitorder="little"
    ).reshape(BATCH, K)
    xbt = np.ascontiguousarray(
        xb.T.reshape(4, 4, 128, BATCH).transpose(0, 2, 1, 3)
    ).astype(_FP8)

    wbits = np.unpackbits(
        w.view(np.uint8).reshape(POP, IN_INTS, 2, OUT_F, 8),
        axis=-1,
        bitorder="little",
    )  # [POP, IN_INTS, 2, OUT_F, 64]
    w0 = wbits[:, :, 0].transpose(0, 1, 3, 2).reshape(POP, K, OUT_F)
    w1 = wbits[:, :, 1].transpose(0, 1, 3, 2).reshape(POP, K, OUT_F)
    bias = w1.sum(axis=1, dtype=np.int32)  # [POP, OUT_F]
    wd = w0.astype(np.int8) - w1.astype(np.int8)  # {-1,0,1}
    wd_cores = [
        np.ascontiguousarray(
            wd[p].reshape(4, 4, 128, 4, 512).transpose(0, 3, 2, 1, 4)
        ).astype(_FP8)
        for p in range(POP)
    ]
    return xbt, wd_cores, bias


def kernel(x, w):
    from concourse.bass_utils import run_bass_kernel_spmd

    nc = get_nc()
    xbt, wd_cores, bias = pack_inputs(np.asarray(x), np.asarray(w))
    in_maps = [{"xbt": xbt, "wd": wd_cores[p]} for p in range(N_CORES)]
    res = run_bass_kernel_spmd(nc, in_maps, list(range(N_CORES)))
    out = np.empty((POP, BATCH, OUT_F), dtype=np.int32)
    for p in range(N_CORES):
        out[p] = res.results[p]["out"].astype(np.int32) + bias[p][None, :]
    return out


# revision 13
# speedup vs baseline: 1.1195x; 1.1195x over previous
"""EvoBinarizedLayer as one fp8 matmul per population member.

Math: per population p, with xb = unpacked bits of x (LSB-first) and
w0/w1 the two unpacked weight bit-planes,

  count[p] = xb @ w0 + (1 - xb) @ w1
           = xb @ (w0 - w1) + colsum(w1)

so each core computes a single [512,2048] @ [2048,2048] matmul with
lhs entries in {0,1} and rhs entries in {-1,0,1} (both exact in fp8
e4m3, accumulated exactly in fp32 PSUM), plus a per-(p,o) bias added
on the host. Counts <= 2048 are exact in fp16, so the device emits
fp16 and the host upcasts to int32.

Sharding: population dim P=8, one member per NeuronCore (x replicated).

Device layout: contraction dim K=2048 split into 16 k-tiles of 128
(partition dim); DoubleRow fp8 matmuls consume k-tile pairs (K=256 per
instruction). Weights are streamed in 16 chunks of (k-quarter x
o-quarter), each chunk contiguous per partition so every DMA is 128
descriptors of 2 KiB, spread round-robin over 4 engine DMA queues.
"""

import numpy as np
import ml_dtypes

POP, BATCH, IN_INTS, OUT_F = 8, 512, 32, 2048
K = IN_INTS * 64          # 2048 contraction (bit) dim
KT = K // 128             # 16 k-tiles of 128
N_CORES = 8

_FP8 = ml_dtypes.float8_e4m3

_cached = {}

# matmul loop order: "A" = (ob, bt) outer, k inner (stationary reloaded
# every matmul); "B" = (bt, k) outer, ob inner (stationary repeated 4x
# consecutively, tests walrus LDWEIGHTS dedupe)
MM_ORDER = "A"


def _build_nc():
    import concourse.tile as tile
    from concourse import bacc, mybir

    dt = mybir.dt
    nc = bacc.Bacc(
        "TRN2", target_bir_lowering=False, debug=False, num_devices=N_CORES
    )
    xbt_d = nc.dram_tensor(
        "xbt", [4, 128, 4, BATCH], dt.float8e4, kind="ExternalInput"
    ).ap()
    wd_d = nc.dram_tensor(
        "wd", [4, 4, 128, 4, 512], dt.float8e4, kind="ExternalInput"
    ).ap()
    out_d = nc.dram_tensor(
        "out", [BATCH, OUT_F], dt.float16, kind="ExternalOutput"
    ).ap()

    with tile.TileContext(nc) as tc:
        with (
            tc.tile_pool(name="xbt", bufs=1) as xbt_pool,
            tc.tile_pool(name="wd", bufs=1) as wd_pool,
            tc.tile_pool(name="outp", bufs=4) as out_pool,
            tc.tile_pool(name="psum", bufs=8, space="PSUM") as psum_pool,
        ):
            engines = [nc.sync, nc.scalar, nc.gpsimd]
            rr = [0]

            def next_engine():
                e = engines[rr[0] % len(engines)]
                rr[0] += 1
                return e

            # xbt_sb[p, k, b]: bit row k*128+p, batch b
            xbt_sb = xbt_pool.tile([128, KT, BATCH], dt.float8e4)
            # wd_sb[p, ob, k, o']: bit row k*128+p, out feature ob*512+o'
            wd_sb = wd_pool.tile([128, 4, KT, 512], dt.float8e4)

            # PE warmup: dummy DoubleRow matmuls on a small zeroed tile so
            # the HAM clock-gate opens (K=8/8) before the real stream
            # starts. Small tile keeps the gating memset cheap (~0.3us).
            warm = xbt_pool.tile([128, 2, 128], dt.float8e4, tag="warm")
            nc.vector.memset(warm[:], 0.0)
            wps = psum_pool.tile([128, 512], dt.float32, tag="ps")
            for _ in range(22):
                nc.tensor.matmul(
                    wps[:, :128],
                    warm[:],
                    warm[:],
                    start=True,
                    stop=True,
                    perf_mode=mybir.MatmulPerfMode.DoubleRow,
                )

            # input DMAs in need-order: the (ob=0) pass consumes chunk
            # pairs (xbt_kq, wd[kq,0]) in kq order; stream those first.
            issue = []
            for kq in range(4):
                issue.append(("x", kq))
                issue.append(("w", kq, 0))
            for ob in range(1, 4):
                for kq in range(4):
                    issue.append(("w", kq, ob))
            for item in issue:
                if item[0] == "x":
                    kq = item[1]
                    next_engine().dma_start(
                        xbt_sb[:, 4 * kq : 4 * (kq + 1), :], xbt_d[kq]
                    )
                else:
                    _, kq, ob = item
                    next_engine().dma_start(
                        wd_sb[:, ob, 4 * kq : 4 * (kq + 1), :], wd_d[kq, ob]
                    )

            # chunk-paced: within each o-quarter, sweep k-pairs in the
            # outer loop across 4 concurrent psum banks (one per batch
            # tile) so each arriving 256 KiB chunk feeds 8 matmuls before
            # the next chunk is needed (compute ramp matches DMA supply).
            # The final o-quarter instead runs tile-serial (k inner) so
            # its psum drains stagger and the last CAST+DMA tail is short.
            def drain(ps, ob, bt):
                ot = out_pool.tile([128, 512], dt.float16, tag="ot", name="ot")
                nc.vector.tensor_copy(ot[:], ps[:])
                next_engine().dma_start(
                    out_d[128 * bt : 128 * (bt + 1), 512 * ob : 512 * (ob + 1)],
                    ot[:],
                )

            for ob in range(3):
                pss = [
                    psum_pool.tile(
                        [128, 512], dt.float32, tag="ps", name=f"ps_{ob}_{bt}"
                    )
                    for bt in range(4)
                ]
                for k in range(KT // 2):
                    for bt in range(4):
                        nc.tensor.matmul(
                            pss[bt][:],
                            xbt_sb[:, 2 * k : 2 * k + 2, 128 * bt : 128 * (bt + 1)],
                            wd_sb[:, ob, 2 * k : 2 * k + 2, :],
                            start=(k == 0),
                            stop=(k == KT // 2 - 1),
                            perf_mode=mybir.MatmulPerfMode.DoubleRow,
                        )
                for bt in range(4):
                    drain(pss[bt], ob, bt)
            for bt in range(4):
                ps = psum_pool.tile([128, 512], dt.float32, tag="ps", name="ps_l")
                for k in range(KT // 2):
                    nc.tensor.matmul(
                        ps[:],
                        xbt_sb[:, 2 * k : 2 * k + 2, 128 * bt : 128 * (bt + 1)],
                        wd_sb[:, 3, 2 * k : 2 * k + 2, :],
                        start=(k == 0),
                        stop=(k == KT // 2 - 1),
                        perf_mode=mybir.MatmulPerfMode.DoubleRow,
                    )
                drain(ps, 3, bt)
    nc.compile()
    return nc


def get_nc():
    if "nc" not in _cached:
        _cached["nc"] = _build_nc()
    return _cached["nc"]


def pack_inputs(x, w):
    """Host-side bit unpack + layout. Returns (xbt, wd_cores, bias).

    xbt: [4, 128, 4, BATCH] fp8; xbt[kq, p, k', b] = bit (4kq+k')*128+p of x[b]
    wd_cores[p]: [4, 4, 128, 4, 512] fp8; [kq, ob, p, k', o'] =
        (w0-w1) at bit row (4kq+k')*128+p, out feature ob*512+o'
    bias: [POP, OUT_F] int32 colsum of w1 bits
    """
    xb = np.unpackbits(
        x.view(np.uint8).reshape(BATCH, IN_INTS, 8), axis=-1, bitorder="little"
    ).reshape(BATCH, K)
    xbt = np.ascontiguousarray(
        xb.T.reshape(4, 4, 128, BATCH).transpose(0, 2, 1, 3)
    ).astype(_FP8)

    wbits = np.unpackbits(
        w.view(np.uint8).reshape(POP, IN_INTS, 2, OUT_F, 8),
        axis=-1,
        bitorder="little",
    )  # [POP, IN_INTS, 2, OUT_F, 64]
    w0 = wbits[:, :, 0].transpose(0, 1, 3, 2).reshape(POP, K, OUT_F)
    w1 = wbits[:, :, 1].transpose(0, 1, 3, 2).reshape(POP, K, OUT_F)
    bias = w1.sum(axis=1, dtype=np.int32)  # [POP, OUT_F]
    wd = w0.astype(np.int8) - w1.astype(np.int8)  # {-1,0,1}
    wd_cores = [
        np.ascontiguousarray(
            wd[p].reshape(4, 4, 128, 4, 512).transpose(0, 3, 2, 1, 4)
        ).astype(_FP8)
        for p in range(POP)
    ]
    return xbt, wd_cores, bias


def kernel(x, w):
    from concourse.bass_utils import run_bass_kernel_spmd

    nc = get_nc()
    xbt, wd_cores, bias = pack_inputs(np.asarray(x), np.asarray(w))
    in_maps = [{"xbt": xbt, "wd": wd_cores[p]} for p in range(N_CORES)]
    res = run_bass_kernel_spmd(nc, in_maps, list(range(N_CORES)))
    out = np.empty((POP, BATCH, OUT_F), dtype=np.int32)
    for p in range(N_CORES):
        out[p] = res.results[p]["out"].astype(np.int32) + bias[p][None, :]
    return out


# revision 16
# speedup vs baseline: 1.1438x; 1.0217x over previous
"""EvoBinarizedLayer as one fp8 matmul per population member.

Math: per population p, with xb = unpacked bits of x (LSB-first) and
w0/w1 the two unpacked weight bit-planes,

  count[p] = xb @ w0 + (1 - xb) @ w1
           = xb @ (w0 - w1) + colsum(w1)

so each core computes a single [512,2048] @ [2048,2048] matmul with
lhs entries in {0,1} and rhs entries in {-1,0,1} (both exact in fp8
e4m3, accumulated exactly in fp32 PSUM), plus a per-(p,o) bias added
on the host. Counts <= 2048 are exact in fp16, so the device emits
fp16 and the host upcasts to int32.

Sharding: population dim P=8, one member per NeuronCore (x replicated).

Device layout: contraction dim K=2048 split into 16 k-tiles of 128
(partition dim); DoubleRow fp8 matmuls consume k-tile pairs (K=256 per
instruction). Weights are streamed in 16 chunks of (k-quarter x
o-quarter), each chunk contiguous per partition so every DMA is 128
descriptors of 2 KiB, spread round-robin over the 3 engine DMA queues
(sync/scalar/gpsimd).
"""

import numpy as np
import ml_dtypes

POP, BATCH, IN_INTS, OUT_F = 8, 512, 32, 2048
K = IN_INTS * 64          # 2048 contraction (bit) dim
KT = K // 128             # 16 k-tiles of 128
N_CORES = 8

_FP8 = ml_dtypes.float8_e4m3

_cached = {}


def _build_nc():
    import concourse.tile as tile
    from concourse import bacc, mybir

    dt = mybir.dt
    nc = bacc.Bacc(
        "TRN2", target_bir_lowering=False, debug=False, num_devices=N_CORES
    )
    xbt_d = nc.dram_tensor(
        "xbt", [4, 128, 4, BATCH], dt.float8e4, kind="ExternalInput"
    ).ap()
    wd_d = nc.dram_tensor(
        "wd", [4, 4, 128, 4, 512], dt.float8e4, kind="ExternalInput"
    ).ap()
    out_d = nc.dram_tensor(
        "out", [BATCH, OUT_F], dt.float16, kind="ExternalOutput"
    ).ap()

    with tile.TileContext(nc) as tc:
        with (
            tc.tile_pool(name="xbt", bufs=1) as xbt_pool,
            tc.tile_pool(name="wd", bufs=1) as wd_pool,
            tc.tile_pool(name="outp", bufs=4) as out_pool,
            tc.tile_pool(name="psum", bufs=8, space="PSUM") as psum_pool,
        ):
            engines = [nc.sync, nc.scalar, nc.gpsimd]
            rr = [0]

            def next_engine():
                e = engines[rr[0] % len(engines)]
                rr[0] += 1
                return e

            # xbt_sb[p, k, b]: bit row k*128+p, batch b
            xbt_sb = xbt_pool.tile([128, KT, BATCH], dt.float8e4)
            # wd_sb[p, ob, k, o']: bit row k*128+p, out feature ob*512+o'
            wd_sb = wd_pool.tile([128, 4, KT, 512], dt.float8e4)

            # PE warmup: dummy DoubleRow matmuls on a small zeroed tile so
            # the HAM clock-gate opens (K=8/8) before the real stream
            # starts. Small tile keeps the gating memset cheap (~0.3us).
            warm = xbt_pool.tile([128, 2, 128], dt.float8e4, tag="warm")
            nc.vector.memset(warm[:], 0.0)
            wps = psum_pool.tile([128, 512], dt.float32, tag="ps")
            for _ in range(22):
                nc.tensor.matmul(
                    wps[:, :128],
                    warm[:],
                    warm[:],
                    start=True,
                    stop=True,
                    perf_mode=mybir.MatmulPerfMode.DoubleRow,
                )

            # input DMAs in need-order: the (ob=0) pass consumes chunk
            # pairs (xbt_kq, wd[kq,0]) in kq order; stream those first.
            issue = []
            for kq in range(4):
                issue.append(("x", kq))
                issue.append(("w", kq, 0))
            for ob in range(1, 4):
                for kq in range(4):
                    issue.append(("w", kq, ob))
            for item in issue:
                if item[0] == "x":
                    kq = item[1]
                    next_engine().dma_start(
                        xbt_sb[:, 4 * kq : 4 * (kq + 1), :], xbt_d[kq]
                    )
                else:
                    _, kq, ob = item
                    next_engine().dma_start(
                        wd_sb[:, ob, 4 * kq : 4 * (kq + 1), :], wd_d[kq, ob]
                    )

            # chunk-paced: within each o-quarter, sweep k-pairs in the
            # outer loop across 4 concurrent psum banks (one per batch
            # tile) so each arriving 256 KiB chunk feeds 8 matmuls before
            # the next chunk is needed (compute ramp matches DMA supply).
            # The final o-quarter instead runs tile-serial (k inner) so
            # its psum drains stagger and the last CAST+DMA tail is short.
            def drain(ps, ob, bt):
                ot = out_pool.tile([128, 512], dt.float16, tag="ot", name="ot")
                nc.vector.tensor_copy(ot[:], ps[:])
                next_engine().dma_start(
                    out_d[128 * bt : 128 * (bt + 1), 512 * ob : 512 * (ob + 1)],
                    ot[:],
                )

            for ob in range(3):
                pss = [
                    psum_pool.tile(
                        [128, 512], dt.float32, tag="ps", name=f"ps_{ob}_{bt}"
                    )
                    for bt in range(4)
                ]
                for k in range(KT // 2):
                    for bt in range(4):
                        nc.tensor.matmul(
                            pss[bt][:],
                            xbt_sb[:, 2 * k : 2 * k + 2, 128 * bt : 128 * (bt + 1)],
                            wd_sb[:, ob, 2 * k : 2 * k + 2, :],
                            start=(k == 0),
                            stop=(k == KT // 2 - 1),
                            perf_mode=mybir.MatmulPerfMode.DoubleRow,
                        )
                for bt in range(4):
                    drain(pss[bt], ob, bt)
            for bt in range(4):
                ps = psum_pool.tile([128, 512], dt.float32, tag="ps", name="ps_l")
                for k in range(KT // 2):
                    nc.tensor.matmul(
                        ps[:],
                        xbt_sb[:, 2 * k : 2 * k + 2, 128 * bt : 128 * (bt + 1)],
                        wd_sb[:, 3, 2 * k : 2 * k + 2, :],
                        start=(k == 0),
                        stop=(k == KT // 2 - 1),
                        perf_mode=mybir.MatmulPerfMode.DoubleRow,
                    )
                drain(ps, 3, bt)
    nc.compile()
    return nc


def get_nc():
    if "nc" not in _cached:
        _cached["nc"] = _build_nc()
    return _cached["nc"]


def pack_inputs(x, w):
    """Host-side bit unpack + layout. Returns (xbt, wd_cores, bias).

    xbt: [4, 128, 4, BATCH] fp8; xbt[kq, p, k', b] = bit (4kq+k')*128+p of x[b]
    wd_cores[p]: [4, 4, 128, 4, 512] fp8; [kq, ob, p, k', o'] =
        (w0-w1) at bit row (4kq+k')*128+p, out feature ob*512+o'
    bias: [POP, OUT_F] int32 colsum of w1 bits
    """
    xb = np.unpackbits(
        x.view(np.uint8).reshape(BATCH, IN_INTS, 8), axis=-1, bitorder="little"
    ).reshape(BATCH, K)
    xbt = np.ascontiguousarray(
        xb.T.reshape(4, 4, 128, BATCH).transpose(0, 2, 1, 3)
    ).astype(_FP8)

    wbits = np.unpackbits(
        w.view(np.uint8).reshape(POP, IN_INTS, 2, OUT_F, 8),
        axis=-1,
        bitorder="little",
    )  # [POP, IN_INTS, 2, OUT_F, 64]
    w0 = wbits[:, :, 0].transpose(0, 1, 3, 2).reshape(POP, K, OUT_F)
    w1 = wbits[:, :, 1].transpose(0, 1, 3, 2).reshape(POP, K, OUT_F)
    bias = w1.sum(axis=1, dtype=np.int32)  # [POP, OUT_F]
    wd = w0.astype(np.int8) - w1.astype(np.int8)  # {-1,0,1}
    wd_cores = [
        np.ascontiguousarray(
            wd[p].reshape(4, 4, 128, 4, 512).transpose(0, 3, 2, 1, 4)
        ).astype(_FP8)
        for p in range(POP)
    ]
    return xbt, wd_cores, bias


def kernel(x, w):
    from concourse.bass_utils import run_bass_kernel_spmd

    nc = get_nc()
    xbt, wd_cores, bias = pack_inputs(np.asarray(x), np.asarray(w))
    in_maps = [{"xbt": xbt, "wd": wd_cores[p]} for p in range(N_CORES)]
    try:
        res = run_bass_kernel_spmd(nc, in_maps, list(range(N_CORES)))
    except Exception:
        # NRT_EXEC_UNIT_UNRECOVERABLE has been observed transiently on this
        # fabric; one retry has always succeeded.
        res = run_bass_kernel_spmd(nc, in_maps, list(range(N_CORES)))
    out = np.empty((POP, BATCH, OUT_F), dtype=np.int32)
    for p in range(N_CORES):
        out[p] = res.results[p]["out"].astype(np.int32) + bias[p][None, :]
    return out
